# revision 10
# baseline (speedup 1.0000x reference)
"""GCN (3-layer, symmetric-normalized, mean-pooled) on 8 Trainium2 NeuronCores.

Strategy:
- Factor the GCN normalization: w[e] = dis[row]*dis[col] with dis = deg^-1/2.
  propagate(h) = dis ⊙ (A @ (dis ⊙ h)), so per-edge weights disappear;
  only per-node scales remain (fused into elementwise passes).
- Shard destination nodes (and their in-edges) across the 8 cores.
- Per layer, per 128-dest tile: dma_gather the source rows g[col] (edge-major),
  reduce via TensorE matmuls against one-hot S matrices built on-device with
  iota==local_dest compares: zT[f,d] += sum_e msg[e,f]*S[e,d]. zT is feat-major,
  which feeds the (z @ W) matmul directly with no transpose.
- AllGather the per-core g shards between layers (ncfw collective).
- Global mean-pool with the same one-hot matmul trick against batch ids.

Host side does only index preprocessing (edge partitioning/padding, int16
gather tables) and the trivial final combine of per-core pool windows.
"""

import math

import numpy as np


def _ceil_div(a, b):
    return (a + b - 1) // b


class _Sched:
    pass


def _preprocess(x, edge_index, batch, n_cores=8):
    """Build the static schedule + per-core tables from the graph indices."""
    N, D = x.shape
    assert D == 128
    assert N % n_cores == 0
    s = _Sched()
    s.N, s.D, s.n_cores = N, D, n_cores
    s.shard = N // n_cores
    s.tiles = _ceil_div(s.shard, 128)
    s.shard_pad = s.tiles * 128
    s.npad = s.shard_pad * n_cores

    row = np.concatenate([np.asarray(edge_index[0]), np.arange(N, dtype=np.int64)])
    col = np.concatenate([np.asarray(edge_index[1]), np.arange(N, dtype=np.int64)])
    deg = np.bincount(row, minlength=N).astype(np.float32)
    dis = deg ** -0.5
    s.dis = dis

    # padded global index (each core's shard padded to shard_pad rows)
    colp = (col // s.shard) * s.shard_pad + (col % s.shard)

    # per (core, tile, parity) edge lists, sorted by core/tile
    core_of = row // s.shard
    tile_of = (row % s.shard) // 128
    parity = colp & 1

    # order edges by (core, tile, parity) with counting sort
    key = (core_of * s.tiles + tile_of) * 2 + parity
    order = np.argsort(key, kind="stable")
    key_s = key[order]
    row_s = row[order]
    colp_s = colp[order]

    nkeys = n_cores * s.tiles * 2
    counts = np.bincount(key_s, minlength=nkeys).reshape(n_cores, s.tiles, 2)
    starts = np.zeros(nkeys + 1, dtype=np.int64)
    np.cumsum(counts.reshape(-1), out=starts[1:])

    # chunk counts per (tile, parity): max over cores (shared static program)
    nch = _ceil_div(counts, 128).max(axis=0)  # [tiles, 2]
    s.nch = nch
    s.totch = int(nch.sum())
    # parity-major global chunk numbering: all parity-0 chunks (tile order),
    # then all parity-1 chunks. Gather calls are rolling groups of <= 8
    # chunks (1024 idx: the SWDGE descriptor ring caps a call at ~65
    # descs/engine) within one parity, crossing tile boundaries freely.
    choff = np.zeros((s.tiles, 2), dtype=np.int64)
    l0 = int(nch[:, 0].sum())
    a0 = a1 = 0
    for t in range(s.tiles):
        choff[t, 0] = a0
        a0 += nch[t, 0]
        choff[t, 1] = l0 + a1
        a1 += nch[t, 1]
    s.choff = choff
    s.plen = (l0, int(nch[:, 1].sum()))
    s.pbase = (0, l0)
    # calls: list of (chunk_base, nchunks, parity)
    s.calls = []
    for p in range(2):
        for j in range(0, s.plen[p], 8):
            s.calls.append((s.pbase[p] + j, min(8, s.plen[p] - j), p))

    # per-core tables
    s.idx_tab = np.zeros((n_cores, 128, 8 * s.totch), dtype=np.int16)
    s.ld_tab = np.full((n_cores, 128, s.totch), -1.0, dtype=np.float32)
    for c in range(n_cores):
        for t in range(s.tiles):
            for p in range(2):
                n = int(nch[t, p])
                if n == 0:
                    continue
                k = c * s.tiles * 2 + t * 2 + p
                lo, hi = starts[k], starts[k + 1]
                cnt = hi - lo
                idx = np.zeros(n * 128, dtype=np.int64)
                idx[:cnt] = colp_s[lo:hi] >> 1
                ld = np.full(n * 128, -1.0, dtype=np.float32)
                ld[:cnt] = (row_s[lo:hi] - c * s.shard - t * 128).astype(np.float32)
                co = int(choff[t, p])
                # idx j -> [j%16, j//16], replicated across the 8 Q7 core groups
                wrapped = idx.astype(np.int16).reshape(-1, 16).T  # [16, n*8]
                s.idx_tab[c, :, 8 * co:8 * (co + n)] = np.tile(wrapped, (8, 1))
                s.ld_tab[c, :, co:co + n] = ld.reshape(n, 128).T

    # per-core dis table (partition = node % 128, col = tile), pad rows -> 0
    s.dis_t = np.zeros((n_cores, 128, s.tiles), dtype=np.float32)
    for c in range(n_cores):
        d = np.zeros(s.shard_pad, dtype=np.float32)
        d[:s.shard] = dis[c * s.shard:(c + 1) * s.shard]
        s.dis_t[c] = d.reshape(s.tiles, 128).T

    # pooling windows: split tiles into nw contiguous groups such that each
    # group's batch-id span is < 128 for every core
    batch = np.asarray(batch)
    s.B = int(batch.max()) + 1 if batch.size else 1
    for nw in range(1, s.tiles + 1):
        bounds = [round(i * s.tiles / nw) for i in range(nw + 1)]
        ok = True
        win_start = np.zeros((n_cores, nw), dtype=np.int64)
        for c in range(n_cores):
            for w in range(nw):
                n0 = c * s.shard + bounds[w] * 128
                n1 = min(c * s.shard + bounds[w + 1] * 128, (c + 1) * s.shard) - 1
                if n0 > n1:
                    win_start[c, w] = 0
                    continue
                b0, b1 = int(batch[n0]), int(batch[n1])
                if b1 - b0 > 127:
                    ok = False
                    break
                win_start[c, w] = b0
            if not ok:
                break
        if ok:
            s.nw = nw
            s.wbounds = bounds
            s.win_start = win_start
            break
    else:
        raise RuntimeError("no pooling window split found")

    # local graph ids per (core, tile): batch[node] - win_start, pad -> -1
    s.lg_tab = np.full((n_cores, 128, s.tiles), -1.0, dtype=np.float32)
    for c in range(n_cores):
        lg = np.full(s.shard_pad, -1.0, dtype=np.float32)
        bshard = batch[c * s.shard:(c + 1) * s.shard].astype(np.float32)
        for w in range(s.nw):
            t0, t1 = s.wbounds[w], s.wbounds[w + 1]
            n0, n1 = t0 * 128, min(t1 * 128, s.shard)
            lg[n0:n1] = bshard[n0:n1] - s.win_start[c, w]
        s.lg_tab[c] = lg.reshape(s.tiles, 128).T

    s.cnts = np.bincount(batch, minlength=s.B).astype(np.float32)

    # host-precomputed one-hot S matrices (bf16), identical for all layers:
    # S[chunk][e, d] = 1 if ld_tab[e, chunk] == d else 0
    try:
        import ml_dtypes
        sdt = ml_dtypes.bfloat16
    except ImportError:
        sdt = np.float32
    s.sdt = sdt
    ar = np.arange(128, dtype=np.float32)
    s.s_tab = np.zeros((n_cores, s.totch, 128, 128), dtype=sdt)
    for c in range(n_cores):
        # [128, totch] ld values -> one-hot along last axis
        ld = s.ld_tab[c]  # [128 (edge), totch]
        oh = (ld[:, :, None] == ar[None, None, :])
        s.s_tab[c] = oh.transpose(1, 0, 2).astype(sdt)
    return s


BF16 = True


def _build(s, layers=3):
    """Build the shared SPMD Bass/Tile program."""
    from contextlib import ExitStack

    import concourse.tile as tile
    from concourse import bacc, mybir

    DT = mybir.dt
    F32 = DT.float32
    GDT = DT.bfloat16 if BF16 else DT.float32
    nc = bacc.Bacc("TRN2", target_bir_lowering=False, debug=False,
                   num_devices=s.n_cores, num_swdge_queues=4)

    g0 = nc.dram_tensor("g0", [s.npad, 128], GDT, kind="ExternalInput")
    w_in = nc.dram_tensor("w_in", [layers, 128, 128], GDT, kind="ExternalInput")
    b_in = nc.dram_tensor("b_in", [layers, 128, 128], F32, kind="ExternalInput")
    iota_in = nc.dram_tensor("iota_in", [128, 128], F32, kind="ExternalInput")
    idx_in = nc.dram_tensor("idx_in", [128, 8 * s.totch], DT.int16, kind="ExternalInput")
    s_in = nc.dram_tensor("s_in", [s.totch, 128, 128], GDT, kind="ExternalInput")
    dis_in = nc.dram_tensor("dis_in", [128, s.tiles], F32, kind="ExternalInput")
    lg_in = nc.dram_tensor("lg_in", [128, s.tiles], F32, kind="ExternalInput")
    pool_out = nc.dram_tensor("pool_out", [s.nw, 128, 128], F32, kind="ExternalOutput")

    g_bounce = nc.dram_tensor("g_bounce", [s.shard_pad, 128], GDT)
    g_full = [
        nc.dram_tensor(f"g_full{l}", [s.npad, 128], GDT, addr_space="Shared")
        for l in range(1, layers)
    ]

    relu = mybir.ActivationFunctionType.Relu
    iseq = mybir.AluOpType.is_equal
    mult = mybir.AluOpType.mult
    add = mybir.AluOpType.add


    with tile.TileContext(nc) as tc, ExitStack() as ctx:
        const = ctx.enter_context(tc.tile_pool(name="const", bufs=1))
        msgp = ctx.enter_context(tc.tile_pool(name="msg", bufs=14))
        sp = ctx.enter_context(tc.tile_pool(name="sp", bufs=14))
        zp = ctx.enter_context(tc.tile_pool(name="zp", bufs=3))
        houtp = ctx.enter_context(tc.tile_pool(name="hout", bufs=1))
        psz = ctx.enter_context(tc.tile_pool(name="psz", bufs=2, space="PSUM"))
        psu = ctx.enter_context(tc.tile_pool(name="psu", bufs=2, space="PSUM"))

        iota_t = const.tile([128, 128], F32)
        nc.sync.dma_start(iota_t[:], iota_in[:])
        idx_t = const.tile([128, 8 * s.totch], DT.int16)
        nc.sync.dma_start(idx_t[:], idx_in[:])
        dis_t = const.tile([128, s.tiles], F32)
        nc.sync.dma_start(dis_t[:], dis_in[:])
        lg_t = const.tile([128, s.tiles], F32)
        nc.sync.dma_start(lg_t[:], lg_in[:])
        w_t = const.tile([128, layers, 128], GDT)
        nc.sync.dma_start(w_t[:], w_in.ap().rearrange("l p d -> p l d"))
        b_t = const.tile([128, layers, 128], F32)
        nc.sync.dma_start(b_t[:], b_in.ap().rearrange("l p d -> p l d"))

        hout = houtp.tile([128, s.tiles * 128], GDT)
        h3 = houtp.tile([128, s.tiles * 128], F32)
        zA = houtp.tile([128, s.tiles * 128], F32)

        for l in range(layers):
            g_src = g0 if l == 0 else g_full[l - 1]
            g_pair = g_src.ap().rearrange("(n two) d -> n (two d)", two=2)

            call_tiles = {}

            def get_msg(gc, g_pair=g_pair, call_tiles=call_tiles):
                p = 0 if gc < s.plen[0] else 1
                loc = gc - s.pbase[p]
                cid = (p, loc // 8)
                if cid not in call_tiles:
                    base = s.pbase[p] + (loc // 8) * 8
                    n = min(8, s.plen[p] - (loc // 8) * 8)
                    m = msgp.tile([128, 8, 128], GDT, tag="msg")
                    qn = len(call_tiles) % 4
                    nc.gpsimd.dma_gather(
                        m[:, 0:n, :],
                        g_pair[:, p * 128:(p + 1) * 128],
                        idx_t[:, 8 * base:8 * (base + n)],
                        n * 128,
                        n * 128,
                        128,
                        elem_step=256,
                        queue_num=qn,
                    )
                    st = sp.tile([128, 8, 128], GDT, tag="S")
                    nc.sync.dma_start(
                        st[:, 0:n, :],
                        s_in.ap()[base:base + n].rearrange("k p f -> p k f"),
                    )
                    call_tiles[cid] = (m, st)
                return call_tiles[cid], loc % 8

            # pass A (parity 0) -> zA; pass B (parity 1) -> combine + post
            for p in range(2):
                for t in range(s.tiles):
                    n = int(s.nch[t, p])
                    zsl = zA[:, t * 128:(t + 1) * 128]
                    pz = None
                    if n > 0:
                        pz = psz.tile([128, 128], F32, tag="pz")
                        for k in range(n):
                            gc = int(s.choff[t, p]) + k
                            (m, st), j = get_msg(gc)
                            nc.tensor.matmul(
                                pz[:], m[:, j, :], st[:, j, :],
                                start=(k == 0), stop=(k == n - 1),
                            )
                    if p == 0:
                        if n > 0:
                            nc.vector.tensor_copy(zsl, pz[:])
                        else:
                            nc.vector.memset(zsl, 0.0)
                        continue

                    zT = zp.tile([128, 128], GDT, tag="zT")
                    if n > 0:
                        nc.vector.tensor_tensor(zT[:], pz[:], zsl, add)
                    else:
                        nc.vector.tensor_copy(zT[:], zsl)
                    pu = psu.tile([128, 128], F32, tag="pu")
                    nc.tensor.matmul(pu[:], zT[:], w_t[:, l, :], start=True, stop=True)
                    t1 = zp.tile([128, 128], F32, tag="t1")
                    nc.vector.tensor_scalar(t1[:], pu[:], dis_t[:, t:t + 1], None, mult)
                    t2 = zp.tile([128, 128], F32, tag="t2")
                    nc.vector.tensor_tensor(t2[:], t1[:], b_t[:, l, :], add)
                    if l < layers - 1:
                        nc.scalar.activation(
                            hout[:, t * 128:(t + 1) * 128], t2[:], relu,
                            scale=dis_t[:, t:t + 1],
                        )
                    else:
                        nc.scalar.activation(
                            h3[:, t * 128:(t + 1) * 128], t2[:], relu
                        )

            if l < layers - 1:
                nc.sync.dma_start(
                    g_bounce.ap().rearrange("(t p) d -> p t d", p=128),
                    hout[:],
                )
                nc.gpsimd.collective_compute(
                    "AllGather",
                    mybir.AluOpType.bypass,
                    replica_groups=[list(range(s.n_cores))],
                    ins=[g_bounce.ap().opt()],
                    outs=[g_full[l].ap().opt()],
                )

        # mean pooling: pool[graph, feat] = sum_node onehot(batch)^T h3
        for w in range(s.nw):
            t0, t1 = s.wbounds[w], s.wbounds[w + 1]
            pz = psz.tile([128, 128], F32, tag="pz")
            for j, t in enumerate(range(t0, t1)):
                P = sp.tile([128, 128], F32, tag="S")
                nc.vector.tensor_scalar(
                    P[:], iota_t[:], lg_t[:, t:t + 1], None, iseq
                )
                nc.tensor.matmul(
                    pz[:], P[:], h3[:, t * 128:(t + 1) * 128],
                    start=(j == 0), stop=(t == t1 - 1),
                )
            pres = zp.tile([128, 128], F32, tag="zT")
            nc.vector.tensor_copy(pres[:], pz[:])
            nc.sync.dma_start(pool_out[w], pres[:])

    nc.compile()
    return nc


def _in_maps(x, Ws, bs, s):
    gdt = np.float32
    if BF16:
        import ml_dtypes
        gdt = ml_dtypes.bfloat16
    g0 = np.zeros((s.npad, 128), dtype=gdt)
    gx = s.dis[:, None] * np.asarray(x, dtype=np.float32)
    for c in range(s.n_cores):
        g0[c * s.shard_pad:c * s.shard_pad + s.shard] = \
            gx[c * s.shard:(c + 1) * s.shard].astype(gdt)

    iota = np.tile(np.arange(128, dtype=np.float32), (128, 1))
    wcat = np.stack([np.asarray(w, np.float32) for w in Ws]).astype(gdt)
    bcat = np.stack([np.tile(np.asarray(b, np.float32), (128, 1)) for b in bs])

    in_maps = []
    for c in range(s.n_cores):
        in_maps.append({
            "g0": g0,
            "w_in": wcat,
            "b_in": bcat,
            "iota_in": iota,
            "idx_in": s.idx_tab[c],
            "s_in": s.s_tab[c],
            "dis_in": s.dis_t[c],
            "lg_in": s.lg_tab[c],
        })

    return in_maps


def _run(x, edge_index, batch, Ws, bs, s, nc):
    from concourse.bass_utils import run_bass_kernel_spmd

    in_maps = _in_maps(x, Ws, bs, s)
    br = run_bass_kernel_spmd(nc, in_maps, list(range(s.n_cores)))

    acc = np.zeros((s.B + 128, 128), dtype=np.float32)
    for c in range(s.n_cores):
        po = br.results[c]["pool_out"]
        for w in range(s.nw):
            ws = int(s.win_start[c, w])
            acc[ws:ws + 128] += po[w]
    out = acc[:s.B] / np.maximum(s.cnts, 1.0)[:, None]
    return out, br


def kernel(x, edge_index, batch, W0, b0, W1, b1, W2, b2):
    x = np.asarray(x)
    edge_index = np.asarray(edge_index)
    batch = np.asarray(batch)
    s = _preprocess(x, edge_index, batch)
    nc = _build(s)
    out, _ = _run(x, edge_index, batch, [W0, W1, W2], [b0, b1, b2], s, nc)
    return out.astype(np.float32)


# revision 11
# speedup vs baseline: 1.0378x; 1.0378x over previous
"""GCN (3-layer, symmetric-normalized, mean-pooled) on 8 Trainium2 NeuronCores.

Strategy:
- Factor the GCN normalization: w[e] = dis[row]*dis[col] with dis = deg^-1/2.
  propagate(h) = dis ⊙ (A @ (dis ⊙ h)), so per-edge weights disappear;
  only per-node scales remain (fused into elementwise passes).
- Shard destination nodes (and their in-edges) across the 8 cores.
- Per layer, per 128-dest tile: dma_gather the source rows g[col] (edge-major),
  reduce via TensorE matmuls against one-hot S matrices built on-device with
  iota==local_dest compares: zT[f,d] += sum_e msg[e,f]*S[e,d]. zT is feat-major,
  which feeds the (z @ W) matmul directly with no transpose.
- AllGather the per-core g shards between layers (ncfw collective).
- Global mean-pool with the same one-hot matmul trick against batch ids.

Host side does only index preprocessing (edge partitioning/padding, int16
gather tables) and the trivial final combine of per-core pool windows.
"""

import math

import numpy as np


def _ceil_div(a, b):
    return (a + b - 1) // b


class _Sched:
    pass


def _preprocess(x, edge_index, batch, n_cores=8):
    """Build the static schedule + per-core tables from the graph indices."""
    N, D = x.shape
    assert D == 128
    assert N % n_cores == 0
    s = _Sched()
    s.N, s.D, s.n_cores = N, D, n_cores
    s.shard = N // n_cores
    s.tiles = _ceil_div(s.shard, 128)
    s.shard_pad = s.tiles * 128
    s.npad = s.shard_pad * n_cores

    row = np.concatenate([np.asarray(edge_index[0]), np.arange(N, dtype=np.int64)])
    col = np.concatenate([np.asarray(edge_index[1]), np.arange(N, dtype=np.int64)])
    deg = np.bincount(row, minlength=N).astype(np.float32)
    dis = deg ** -0.5
    s.dis = dis

    # padded global index (each core's shard padded to shard_pad rows)
    colp = (col // s.shard) * s.shard_pad + (col % s.shard)

    # per (core, tile, parity) edge lists, sorted by core/tile
    core_of = row // s.shard
    tile_of = (row % s.shard) // 128
    parity = colp & 1

    # order edges by (core, tile, parity) with counting sort
    key = (core_of * s.tiles + tile_of) * 2 + parity
    order = np.argsort(key, kind="stable")
    key_s = key[order]
    row_s = row[order]
    colp_s = colp[order]

    nkeys = n_cores * s.tiles * 2
    counts = np.bincount(key_s, minlength=nkeys).reshape(n_cores, s.tiles, 2)
    starts = np.zeros(nkeys + 1, dtype=np.int64)
    np.cumsum(counts.reshape(-1), out=starts[1:])

    # chunk counts per (tile, parity): max over cores (shared static program)
    nch = _ceil_div(counts, 128).max(axis=0)  # [tiles, 2]
    s.nch = nch
    s.totch = int(nch.sum())
    # parity-major global chunk numbering: all parity-0 chunks (tile order),
    # then all parity-1 chunks. Gather calls are rolling groups of <= 8
    # chunks (1024 idx: the SWDGE descriptor ring caps a call at ~65
    # descs/engine) within one parity, crossing tile boundaries freely.
    choff = np.zeros((s.tiles, 2), dtype=np.int64)
    l0 = int(nch[:, 0].sum())
    a0 = a1 = 0
    for t in range(s.tiles):
        choff[t, 0] = a0
        a0 += nch[t, 0]
        choff[t, 1] = l0 + a1
        a1 += nch[t, 1]
    s.choff = choff
    s.plen = (l0, int(nch[:, 1].sum()))
    s.pbase = (0, l0)
    # calls: list of (chunk_base, nchunks, parity)
    s.calls = []
    for p in range(2):
        for j in range(0, s.plen[p], 8):
            s.calls.append((s.pbase[p] + j, min(8, s.plen[p] - j), p))

    # per-core tables
    s.idx_tab = np.zeros((n_cores, 128, 8 * s.totch), dtype=np.int16)
    s.ld_tab = np.full((n_cores, 128, s.totch), -1.0, dtype=np.float32)
    for c in range(n_cores):
        for t in range(s.tiles):
            for p in range(2):
                n = int(nch[t, p])
                if n == 0:
                    continue
                k = c * s.tiles * 2 + t * 2 + p
                lo, hi = starts[k], starts[k + 1]
                cnt = hi - lo
                idx = np.zeros(n * 128, dtype=np.int64)
                idx[:cnt] = colp_s[lo:hi] >> 1
                ld = np.full(n * 128, -1.0, dtype=np.float32)
                ld[:cnt] = (row_s[lo:hi] - c * s.shard - t * 128).astype(np.float32)
                co = int(choff[t, p])
                # idx j -> [j%16, j//16], replicated across the 8 Q7 core groups
                wrapped = idx.astype(np.int16).reshape(-1, 16).T  # [16, n*8]
                s.idx_tab[c, :, 8 * co:8 * (co + n)] = np.tile(wrapped, (8, 1))
                s.ld_tab[c, :, co:co + n] = ld.reshape(n, 128).T

    # per-core dis table (partition = node % 128, col = tile), pad rows -> 0
    s.dis_t = np.zeros((n_cores, 128, s.tiles), dtype=np.float32)
    for c in range(n_cores):
        d = np.zeros(s.shard_pad, dtype=np.float32)
        d[:s.shard] = dis[c * s.shard:(c + 1) * s.shard]
        s.dis_t[c] = d.reshape(s.tiles, 128).T

    # pooling windows: split tiles into nw contiguous groups such that each
    # group's batch-id span is < 128 for every core
    batch = np.asarray(batch)
    s.B = int(batch.max()) + 1 if batch.size else 1
    for nw in range(1, s.tiles + 1):
        bounds = [round(i * s.tiles / nw) for i in range(nw + 1)]
        ok = True
        win_start = np.zeros((n_cores, nw), dtype=np.int64)
        for c in range(n_cores):
            for w in range(nw):
                n0 = c * s.shard + bounds[w] * 128
                n1 = min(c * s.shard + bounds[w + 1] * 128, (c + 1) * s.shard) - 1
                if n0 > n1:
                    win_start[c, w] = 0
                    continue
                b0, b1 = int(batch[n0]), int(batch[n1])
                if b1 - b0 > 127:
                    ok = False
                    break
                win_start[c, w] = b0
            if not ok:
                break
        if ok:
            s.nw = nw
            s.wbounds = bounds
            s.win_start = win_start
            break
    else:
        raise RuntimeError("no pooling window split found")

    # local graph ids per (core, tile): batch[node] - win_start, pad -> -1
    s.lg_tab = np.full((n_cores, 128, s.tiles), -1.0, dtype=np.float32)
    for c in range(n_cores):
        lg = np.full(s.shard_pad, -1.0, dtype=np.float32)
        bshard = batch[c * s.shard:(c + 1) * s.shard].astype(np.float32)
        for w in range(s.nw):
            t0, t1 = s.wbounds[w], s.wbounds[w + 1]
            n0, n1 = t0 * 128, min(t1 * 128, s.shard)
            lg[n0:n1] = bshard[n0:n1] - s.win_start[c, w]
        s.lg_tab[c] = lg.reshape(s.tiles, 128).T

    s.cnts = np.bincount(batch, minlength=s.B).astype(np.float32)

    # host-precomputed one-hot S matrices (bf16), identical for all layers:
    # S[chunk][e, d] = 1 if ld_tab[e, chunk] == d else 0
    try:
        import ml_dtypes
        sdt = ml_dtypes.bfloat16
    except ImportError:
        sdt = np.float32
    s.sdt = sdt
    ar = np.arange(128, dtype=np.float32)
    s.s_tab = np.zeros((n_cores, s.totch, 128, 128), dtype=sdt)
    for c in range(n_cores):
        # [128, totch] ld values -> one-hot along last axis
        ld = s.ld_tab[c]  # [128 (edge), totch]
        oh = (ld[:, :, None] == ar[None, None, :])
        s.s_tab[c] = oh.transpose(1, 0, 2).astype(sdt)
    return s


BF16 = True


def _build(s, layers=3):
    """Build the shared SPMD Bass/Tile program."""
    from contextlib import ExitStack

    import concourse.tile as tile
    from concourse import bacc, mybir

    DT = mybir.dt
    F32 = DT.float32
    GDT = DT.bfloat16 if BF16 else DT.float32
    nc = bacc.Bacc("TRN2", target_bir_lowering=False, debug=False,
                   num_devices=s.n_cores, num_swdge_queues=4)

    g0 = nc.dram_tensor("g0", [s.npad, 128], GDT, kind="ExternalInput")
    w_in = nc.dram_tensor("w_in", [layers, 128, 128], GDT, kind="ExternalInput")
    b_in = nc.dram_tensor("b_in", [layers, 128, 128], F32, kind="ExternalInput")
    iota_in = nc.dram_tensor("iota_in", [128, 128], F32, kind="ExternalInput")
    idx_in = nc.dram_tensor("idx_in", [128, 8 * s.totch], DT.int16, kind="ExternalInput")
    s_in = nc.dram_tensor("s_in", [s.totch, 128, 128], GDT, kind="ExternalInput")
    dis_in = nc.dram_tensor("dis_in", [128, s.tiles], F32, kind="ExternalInput")
    lg_in = nc.dram_tensor("lg_in", [128, s.tiles], F32, kind="ExternalInput")
    pool_out = nc.dram_tensor("pool_out", [s.nw, 128, 128], F32, kind="ExternalOutput")

    g_bounce = nc.dram_tensor("g_bounce", [s.shard_pad, 128], GDT)
    g_full = [
        nc.dram_tensor(f"g_full{l}", [s.npad, 128], GDT, addr_space="Shared")
        for l in range(1, layers)
    ]

    relu = mybir.ActivationFunctionType.Relu
    iseq = mybir.AluOpType.is_equal
    mult = mybir.AluOpType.mult
    add = mybir.AluOpType.add


    with tile.TileContext(nc) as tc, ExitStack() as ctx:
        const = ctx.enter_context(tc.tile_pool(name="const", bufs=1))
        msgp = ctx.enter_context(tc.tile_pool(name="msg", bufs=6))
        sp = ctx.enter_context(tc.tile_pool(name="sp", bufs=4))
        zp = ctx.enter_context(tc.tile_pool(name="zp", bufs=3))
        houtp = ctx.enter_context(tc.tile_pool(name="hout", bufs=1))
        psz = ctx.enter_context(tc.tile_pool(name="psz", bufs=2, space="PSUM"))
        psu = ctx.enter_context(tc.tile_pool(name="psu", bufs=2, space="PSUM"))

        iota_t = const.tile([128, 128], F32)
        nc.sync.dma_start(iota_t[:], iota_in[:])
        idx_t = const.tile([128, 8 * s.totch], DT.int16)
        nc.sync.dma_start(idx_t[:], idx_in[:])
        dis_t = const.tile([128, s.tiles], F32)
        nc.sync.dma_start(dis_t[:], dis_in[:])
        lg_t = const.tile([128, s.tiles], F32)
        nc.sync.dma_start(lg_t[:], lg_in[:])
        w_t = const.tile([128, layers, 128], GDT)
        nc.sync.dma_start(w_t[:], w_in.ap().rearrange("l p d -> p l d"))
        b_t = const.tile([128, layers, 128], F32)
        nc.sync.dma_start(b_t[:], b_in.ap().rearrange("l p d -> p l d"))

        hout = houtp.tile([128, s.tiles * 128], GDT)
        h3 = houtp.tile([128, s.tiles * 128], F32)
        zA = houtp.tile([128, s.tiles * 128], F32)

        for l in range(layers):
            g_src = g0 if l == 0 else g_full[l - 1]
            g_pair = g_src.ap().rearrange("(n two) d -> n (two d)", two=2)

            call_tiles = {}

            def get_msg(gc, g_pair=g_pair, call_tiles=call_tiles):
                p = 0 if gc < s.plen[0] else 1
                loc = gc - s.pbase[p]
                cid = (p, loc // 8)
                if cid not in call_tiles:
                    base = s.pbase[p] + (loc // 8) * 8
                    n = min(8, s.plen[p] - (loc // 8) * 8)
                    m = msgp.tile([128, 8, 128], GDT, tag="msg")
                    qn = len(call_tiles) % 4
                    nc.gpsimd.dma_gather(
                        m[:, 0:n, :],
                        g_pair[:, p * 128:(p + 1) * 128],
                        idx_t[:, 8 * base:8 * (base + n)],
                        n * 128,
                        n * 128,
                        128,
                        elem_step=256,
                        queue_num=qn,
                    )
                    st = sp.tile([128, 8, 128], GDT, tag="S")
                    nc.sync.dma_start(
                        st[:, 0:n, :],
                        s_in.ap()[base:base + n].rearrange("k p f -> p k f"),
                    )
                    call_tiles[cid] = (m, st)
                return call_tiles[cid], loc % 8

            # pass A (parity 0) -> zA; pass B (parity 1) -> combine + post
            for p in range(2):
                for t in range(s.tiles):
                    n = int(s.nch[t, p])
                    zsl = zA[:, t * 128:(t + 1) * 128]
                    pz = None
                    if n > 0:
                        pz = psz.tile([128, 128], F32, tag="pz")
                        for k in range(n):
                            gc = int(s.choff[t, p]) + k
                            (m, st), j = get_msg(gc)
                            nc.tensor.matmul(
                                pz[:], m[:, j, :], st[:, j, :],
                                start=(k == 0), stop=(k == n - 1),
                            )
                    if p == 0:
                        if n > 0:
                            nc.vector.tensor_copy(zsl, pz[:])
                        else:
                            nc.vector.memset(zsl, 0.0)
                        continue

                    zT = zp.tile([128, 128], GDT, tag="zT")
                    if n > 0:
                        nc.vector.tensor_tensor(zT[:], pz[:], zsl, add)
                    else:
                        nc.vector.tensor_copy(zT[:], zsl)
                    pu = psu.tile([128, 128], F32, tag="pu")
                    nc.tensor.matmul(pu[:], zT[:], w_t[:, l, :], start=True, stop=True)
                    t1 = zp.tile([128, 128], F32, tag="t1")
                    nc.vector.tensor_scalar(t1[:], pu[:], dis_t[:, t:t + 1], None, mult)
                    t2 = zp.tile([128, 128], F32, tag="t2")
                    nc.vector.tensor_tensor(t2[:], t1[:], b_t[:, l, :], add)
                    if l < layers - 1:
                        nc.scalar.activation(
                            hout[:, t * 128:(t + 1) * 128], t2[:], relu,
                            scale=dis_t[:, t:t + 1],
                        )
                    else:
                        nc.scalar.activation(
                            h3[:, t * 128:(t + 1) * 128], t2[:], relu
                        )

            if l < layers - 1:
                nc.sync.dma_start(
                    g_bounce.ap().rearrange("(t p) d -> p t d", p=128),
                    hout[:],
                )
                nc.gpsimd.collective_compute(
                    "AllGather",
                    mybir.AluOpType.bypass,
                    replica_groups=[list(range(s.n_cores))],
                    ins=[g_bounce.ap().opt()],
                    outs=[g_full[l].ap().opt()],
                )

        # mean pooling: pool[graph, feat] = sum_node onehot(batch)^T h3
        for w in range(s.nw):
            t0, t1 = s.wbounds[w], s.wbounds[w + 1]
            pz = psz.tile([128, 128], F32, tag="pz")
            for j, t in enumerate(range(t0, t1)):
                P = sp.tile([128, 128], F32, tag="S")
                nc.vector.tensor_scalar(
                    P[:], iota_t[:], lg_t[:, t:t + 1], None, iseq
                )
                nc.tensor.matmul(
                    pz[:], P[:], h3[:, t * 128:(t + 1) * 128],
                    start=(j == 0), stop=(t == t1 - 1),
                )
            pres = zp.tile([128, 128], F32, tag="zT")
            nc.vector.tensor_copy(pres[:], pz[:])
            nc.sync.dma_start(pool_out[w], pres[:])

    nc.compile()
    return nc


def _in_maps(x, Ws, bs, s):
    gdt = np.float32
    if BF16:
        import ml_dtypes
        gdt = ml_dtypes.bfloat16
    g0 = np.zeros((s.npad, 128), dtype=gdt)
    gx = s.dis[:, None] * np.asarray(x, dtype=np.float32)
    for c in range(s.n_cores):
        g0[c * s.shard_pad:c * s.shard_pad + s.shard] = \
            gx[c * s.shard:(c + 1) * s.shard].astype(gdt)

    iota = np.tile(np.arange(128, dtype=np.float32), (128, 1))
    wcat = np.stack([np.asarray(w, np.float32) for w in Ws]).astype(gdt)
    bcat = np.stack([np.tile(np.asarray(b, np.float32), (128, 1)) for b in bs])

    in_maps = []
    for c in range(s.n_cores):
        in_maps.append({
            "g0": g0,
            "w_in": wcat,
            "b_in": bcat,
            "iota_in": iota,
            "idx_in": s.idx_tab[c],
            "s_in": s.s_tab[c],
            "dis_in": s.dis_t[c],
            "lg_in": s.lg_tab[c],
        })

    return in_maps


def _run(x, edge_index, batch, Ws, bs, s, nc):
    from concourse.bass_utils import run_bass_kernel_spmd

    in_maps = _in_maps(x, Ws, bs, s)
    br = run_bass_kernel_spmd(nc, in_maps, list(range(s.n_cores)))

    acc = np.zeros((s.B + 128, 128), dtype=np.float32)
    for c in range(s.n_cores):
        po = br.results[c]["pool_out"]
        for w in range(s.nw):
            ws = int(s.win_start[c, w])
            acc[ws:ws + 128] += po[w]
    out = acc[:s.B] / np.maximum(s.cnts, 1.0)[:, None]
    return out, br


def kernel(x, edge_index, batch, W0, b0, W1, b1, W2, b2):
    x = np.asarray(x)
    edge_index = np.asarray(edge_index)
    batch = np.asarray(batch)
    s = _preprocess(x, edge_index, batch)
    nc = _build(s)
    out, _ = _run(x, edge_index, batch, [W0, W1, W2], [b0, b1, b2], s, nc)
    return out.astype(np.float32)
